# revision 1
# baseline (speedup 1.0000x reference)
"""Trainium2 Bass kernel for causal self-attention (nn_Casualselfatt).

Reference computes (B=2, S=2048, E=1024, H=16, D=64, fp32):
    qkv = x @ W_qkv + b_qkv ; q,k,v = split(qkv)
    q = q.reshape(B, H, S, D)   # NOTE: raw reshape, no transpose.
    ...causal softmax attention per (b,h)...
    out = res @ W_proj + b_proj

The raw reshape means head h of batch b attends over the [S, D] reshape of
rows [128h, 128h+128) of q/k/v[b].  Sharding: 32 (b,h) pairs -> 4 heads of
one batch per core (core c: b=c//4, heads 4*(c%4)..+4).  Each core computes
a partial projection output; the host sums 4 partials per batch.

On-chip: scores are built transposed ([k-part, q-free]) so the softmax
denominator rides an appended ones-column through the AV matmul.  QKV runs
in bf16 (fp32 accumulate); scores run in float32r (Q/K rounded from the
fp32 psum); the post-softmax path (att weights, V, res, W_proj) is bf16.
Phase 1 is split by head-pair so the second pair's QKV matmuls (PE) overlap
the first pair's softmax exp (ACT).  K rows within each 128-block are
stored parity-permuted (sigma: pos = 64*(c%2) + 8*(c//2) + rho for
t = 16*rho + c) so V's k-major blocks can be built with cheap full-width
copies + PE transposes; the causal mask rows are permuted to match.
"""

import numpy as np
import ml_dtypes

import concourse.bass as bass
import concourse.tile as tile
from concourse import bacc, mybir
import concourse.bass_utils as bass_utils

F32 = mybir.dt.float32
F32R = mybir.dt.float32r
BF16 = mybir.dt.bfloat16

B, S, E = 2, 2048, 1024
H, D = 16, 64
N_CORES = 8
HEADS_PER_CORE = 4
ROWS = 128 * HEADS_PER_CORE  # x rows per core
NM = 24                      # qkv column chunks of 128 (q:0-7, k:8-15, v:16-23)
KT = 8                       # contraction tiles over E
NG = 4                       # q groups of 512
NB = S // 128                # 16 blocks of 128 along s'


def sigma_t_of_pos(pos):
    """k-position stored at slot `pos` within a 128-block (parity-split)."""
    par, rem = divmod(pos, 64)
    gam, rho = divmod(rem, 8)
    return 16 * rho + 2 * gam + par


def build_program(with_qkv_bias: bool, repeat: int = 1, phases: int = 7):
    nc = bacc.Bacc("TRN2", target_bir_lowering=False, debug=False,
                   num_devices=N_CORES)

    xin = nc.dram_tensor("xin", [HEADS_PER_CORE, 128, E], F32, kind="ExternalInput")
    wqkv = nc.dram_tensor("wqkv", [NM, 128, KT, 128], BF16, kind="ExternalInput")
    wproj = nc.dram_tensor("wproj", [2, 128, E], BF16, kind="ExternalInput")
    ident_in = nc.dram_tensor("ident", [128, 128], F32, kind="ExternalInput")
    identb_in = nc.dram_tensor("identb", [128, 64], BF16, kind="ExternalInput")
    triu_in = nc.dram_tensor("triu", [128, 128], BF16, kind="ExternalInput")
    if with_qkv_bias:
        bqkv = nc.dram_tensor("bqkv", [128, NM], F32, kind="ExternalInput")
    out = nc.dram_tensor("out", [S, E], F32, kind="ExternalOutput")

    with tile.TileContext(nc) as tc:
        with (
            tc.tile_pool(name="const", bufs=1) as constp,
            tc.tile_pool(name="persist", bufs=1) as persist,
        ):
            ident = constp.tile([128, 128], F32)
            nc.sync.dma_start(ident[:], ident_in.ap())
            identb = constp.tile([128, 64], BF16)
            nc.sync.dma_start(identb[:], identb_in.ap())
            triu = constp.tile([128, 128], BF16)
            nc.sync.dma_start(triu[:], triu_in.ap())
            if with_qkv_bias:
                bias_sb = constp.tile([128, NM], F32)
                nc.sync.dma_start(bias_sb[:], bqkv.ap())

            wp_sb = [persist.tile([128, E], BF16, tag=f"wp{i}", name=f"wp{i}")
                     for i in range(2)]
            for i in range(2):
                nc.sync.dma_start(wp_sb[i][:], wproj.ap()[i])

            # xT[p, kt, r]: x rows (4 heads * 128) transposed, bf16
            xT = persist.tile([128, KT, ROWS], BF16, tag="xT")
            # Q/K transposed per head-pair: [128 (2 heads x 64 d), 2048 (s')]
            qt = [persist.tile([128, S], F32R, tag=f"qt{i}", name=f"qt{i}")
                  for i in range(2)]
            kt_ = [persist.tile([128, S], F32R, tag=f"kt{i}", name=f"ktt{i}")
                   for i in range(2)]
            # V natural per head: 16 blocks of [128, 65] (col 64 = ones),
            # k rows sigma-permuted within each block
            vnat = [persist.tile([128, NB * 65], BF16, tag=f"vn{i}", name=f"vn{i}")
                    for i in range(4)]
            # res^T per head-pair (normalized), bf16
            res = [persist.tile([128, S], BF16, tag=f"res{i}", name=f"res{i}")
                   for i in range(2)]

            # ---- phase 0: transpose x ----
            with (
                tc.tile_pool(name="xn", bufs=2) as xnp,
                tc.tile_pool(name="trps", bufs=2, space="PSUM") as trps,
            ):
                for t in range(HEADS_PER_CORE):
                    xn = xnp.tile([128, E], F32)
                    nc.sync.dma_start(xn[:], xin.ap()[t])
                    for j in range(KT):
                        tp = trps.tile([128, 128], F32)
                        nc.tensor.transpose(tp[:], xn[:, 128 * j:128 * j + 128],
                                            ident[:])
                        nc.vector.tensor_copy(xT[:, j, 128 * t:128 * t + 128],
                                              tp[:])

            def add_bias(dst_ap, src_ap, m, partn=64):
                base = src_ap.base_partition()
                nc.vector.tensor_scalar_add(
                    dst_ap, src_ap, bias_sb[base:base + partn, m:m + 1])

            def chunk_copy(dst_ap, src_ap, m, partn=64):
                if with_qkv_bias:
                    add_bias(dst_ap, src_ap, m, partn)
                else:
                    nc.vector.tensor_copy(dst_ap, src_ap)

            # ---- per head-pair: qkv projection, then attention ----
            with (
                tc.tile_pool(name="wch", bufs=4) as wch,
                tc.tile_pool(name="qkvps", bufs=2, space="PSUM") as qkvps,
                tc.tile_pool(name="vta", bufs=1) as vtap,
                tc.tile_pool(name="scps", bufs=2, space="PSUM") as scps,
                tc.tile_pool(name="avps", bufs=2, space="PSUM") as avps,
                tc.tile_pool(name="expp", bufs=8) as expp,
                tc.tile_pool(name="nrm", bufs=4) as nrm,
            ):
                vt = [vtap.tile([64, S], BF16, tag=f"vt{i}",
                                name=f"vt{i}") for i in range(4)]

                def qkv_phase(hp):
                    r0 = 256 * hp  # column offset of this head-pair in xT
                    for m in range(NM):
                        w = wch.tile([128, KT, 128], BF16)
                        nc.sync.dma_start(w[:], wqkv.ap()[m])
                        ps = qkvps.tile([128, 256], F32)
                        for j in range(KT):
                            nc.tensor.matmul(ps[:], w[:, j, :],
                                             xT[:, j, r0:r0 + 256],
                                             start=(j == 0), stop=(j == KT - 1))
                        if m < 16:
                            # q chunks natural-strided; k chunks sigma-strided
                            dest = qt if m < 8 else kt_
                            mm = m if m < 8 else m - 8
                            for pos in range(2):  # head within pair
                                dstt = dest[hp]
                                rblk = ps[:, 128 * pos:128 * pos + 128]
                                dv = dstt[:].rearrange("p (r c) -> p r c",
                                                       c=16)
                                chunk_copy(
                                    dv[64 * pos:64 * pos + 64, :, 2 * mm],
                                    rblk[0:64, :], m)
                                chunk_copy(
                                    dv[64 * pos:64 * pos + 64, :, 2 * mm + 1],
                                    rblk[64:128, :], m)
                        else:
                            mm = m - 16
                            for pos in range(2):
                                head = 2 * hp + pos
                                dv = vt[head][:].rearrange("p (r c) -> p r c",
                                                           c=16)
                                rblk = ps[:, 128 * pos:128 * pos + 128]
                                chunk_copy(dv[:, :, 2 * mm], rblk[0:64, :], m)
                                chunk_copy(dv[:, :, 2 * mm + 1], rblk[64:128, :],
                                           m)

                def vtrans_phase(hp):
                    # V blocks: transpose [64, 128] slices to natural order
                    for pos in range(2):
                        head = 2 * hp + pos
                        for jj in range(NB):
                            vp = qkvps.tile([128, 256], BF16, tag="ps",
                                            name=f"vp{head}_{jj}")
                            nc.tensor.transpose(
                                vp[:, 0:64],
                                vt[head][:, 128 * jj:128 * jj + 128],
                                identb[0:64, :])
                            nc.vector.tensor_copy(
                                vnat[head][:, 65 * jj:65 * jj + 64],
                                vp[:, 0:64])
                        nc.vector.memset(
                            vnat[head][:].rearrange(
                                "p (jj c) -> p jj c", c=65)[:, :, 64], 1.0)

                def attn_g(hp, g):
                        q0 = 512 * g
                        av = [avps.tile([65, 512], F32, tag="av",
                                        name=f"av{hp}_{g}_{i}") for i in range(2)]
                        nkb = 4 * g + 4
                        for kb in range(nkb):
                            ingroup = kb >= 4 * g
                            coff = 128 * (kb - 4 * g) if ingroup else 0
                            sc = scps.tile([128, 1024], F32, tag="sc")
                            ex = expp.tile([128, 1024], BF16, tag="ex")
                            for pos in range(2):
                                so = 512 * pos
                                nc.tensor.matmul(
                                    sc[:, so + coff:so + 512],
                                    kt_[hp][64 * pos:64 * pos + 64,
                                            128 * kb:128 * kb + 128],
                                    qt[hp][64 * pos:64 * pos + 64,
                                           q0 + coff:q0 + 512],
                                    start=True, stop=True,
                                    tile_position=(64 * pos, 0))
                            if not ingroup:
                                nc.scalar.activation(
                                    ex[:], sc[:],
                                    mybir.ActivationFunctionType.Exp,
                                    scale=0.125)
                            else:
                                sc3 = sc[:].rearrange("p (s q) -> p s q", s=2)
                                ex3 = ex[:].rearrange("p (s q) -> p s q", s=2)
                                nc.scalar.activation(
                                    ex3[:, :, coff:512],
                                    sc3[:, :, coff:512],
                                    mybir.ActivationFunctionType.Exp,
                                    scale=0.125)
                                for pos in range(2):
                                    so = 512 * pos
                                    nc.vector.tensor_mul(
                                        ex[:, so + coff:so + coff + 128],
                                        ex[:, so + coff:so + coff + 128],
                                        triu[:])
                            for pos in range(2):
                                so = 512 * pos
                                head = 2 * hp + pos
                                nc.tensor.matmul(
                                    av[pos][:, coff:512],
                                    vnat[head][:, 65 * kb:65 * kb + 65],
                                    ex[:, so + coff:so + 512],
                                    start=(kb == 0), stop=(kb == nkb - 1))
                        for pos in range(2):
                            rec = nrm.tile([1, 512], F32, tag="rec")
                            nc.vector.reciprocal(rec[:], av[pos][64:65, :])
                            bc = nrm.tile([64, 512], F32, tag="bc")
                            nc.gpsimd.partition_broadcast(bc[:], rec[:])
                            nc.vector.tensor_mul(
                                res[hp][64 * pos:64 * pos + 64, q0:q0 + 512],
                                av[pos][0:64, :], bc[:])

                def proj_group(gg, osb):
                    for blk in range(4 * gg, 4 * gg + 4):
                        o = osb.tile([128, E], F32, name=f"o{blk}", tag="o")
                        for f in range(2):
                            pp = qkvps.tile([128, 512], F32, tag="ps",
                                            name=f"pp{blk}_{f}")
                            for hp in range(2):
                                nc.tensor.matmul(
                                    pp[:], res[hp][:, 128 * blk:128 * blk + 128],
                                    wp_sb[hp][:, 512 * f:512 * f + 512],
                                    start=(hp == 0), stop=(hp == 1))
                            if f == 0:
                                nc.vector.tensor_copy(o[:, 0:512], pp[:])
                            else:
                                nc.scalar.copy(o[:, 512:1024], pp[:])
                        nc.sync.dma_start(
                            out.ap()[128 * blk:128 * blk + 128, :], o[:])

                def body():
                    if phases & 1:
                        qkv_phase(0)
                        vtrans_phase(0)
                    if phases & 2:
                        for g in range(NG):
                            attn_g(0, g)
                    if phases & 1:
                        qkv_phase(1)
                        vtrans_phase(1)
                    for g in range(NG):
                        if phases & 2:
                            attn_g(1, g)
                        if phases & 4:
                            proj_group(g, osb)

                with tc.tile_pool(name="osb", bufs=3) as osb:
                    if repeat == 1:
                        body()
                    else:
                        with tc.For_i(0, repeat, 1):
                            body()

    nc.compile()
    return nc


_CACHE = {}


def _get_program(with_qkv_bias: bool):
    if with_qkv_bias not in _CACHE:
        _CACHE[with_qkv_bias] = build_program(with_qkv_bias)
    return _CACHE[with_qkv_bias]


def make_in_maps(x, W_qkv, b_qkv, W_proj):
    """Build the 8 per-core input maps (host-side data marshaling only)."""
    x = np.ascontiguousarray(np.asarray(x, dtype=np.float32))
    W_qkv = np.asarray(W_qkv, dtype=np.float32)
    b_qkv = np.asarray(b_qkv, dtype=np.float32)
    W_proj = np.asarray(W_proj, dtype=np.float32)

    wq_t = np.ascontiguousarray(
        W_qkv.astype(ml_dtypes.bfloat16).reshape(KT, 128, NM, 128)
        .transpose(2, 1, 0, 3))
    wp_b = W_proj.astype(ml_dtypes.bfloat16)
    ident = np.eye(128, dtype=np.float32)
    identb = np.vstack([np.eye(64), np.eye(64)]).astype(ml_dtypes.bfloat16)
    # causal mask for diagonal blocks, rows sigma-permuted: visible k<=q
    triu = np.triu(np.ones((128, 128), np.float32)).astype(ml_dtypes.bfloat16)
    with_bias = bool(np.any(b_qkv))
    bias_t = np.ascontiguousarray(b_qkv.reshape(NM, 128).T) if with_bias else None

    in_maps = []
    for c in range(N_CORES):
        b, qi = c // 4, c % 4
        m = {
            "xin": np.ascontiguousarray(
                x[b, ROWS * qi:ROWS * qi + ROWS, :].reshape(
                    HEADS_PER_CORE, 128, E)),
            "wqkv": wq_t,
            "wproj": np.ascontiguousarray(
                wp_b[256 * qi:256 * qi + 256, :].reshape(2, 128, E)),
            "ident": ident,
            "identb": identb,
            "triu": triu,
        }
        if with_bias:
            m["bqkv"] = bias_t
        in_maps.append(m)
    return in_maps, with_bias


def kernel(x, W_qkv, b_qkv, W_proj, b_proj, _run_kwargs=None):
    in_maps, with_bias = make_in_maps(x, W_qkv, b_qkv, W_proj)
    nc = _get_program(with_bias)
    res = bass_utils.run_bass_kernel_spmd(
        nc, in_maps, core_ids=list(range(N_CORES)), **(_run_kwargs or {}))
    out = np.zeros((B, S, E), np.float32)
    for c in range(N_CORES):
        out[c // 4] += res.results[c]["out"]
    out += np.asarray(b_proj, dtype=np.float32)[None, None, :]
    if _run_kwargs:
        kernel.last_results = res
    return out

